# revision 7
# baseline (speedup 1.0000x reference)
"""Trainium2 Bass kernel for the FFE module.

Sharding: pure data parallelism — sample i of the batch (N=8) runs on core i.

Per-core layout: 128 SBUF partitions = (row-half hh in {0,1}) x (channel c in 0..63),
partition index hh*64 + c. Each partition holds a reflection-padded sub-image of its
channel: 66 rows x 130 cols (its 64 output rows plus a 1-row/1-col halo).

The dynamic 3x3 depthwise conv (softmax filter from GAP) runs on the tensor engine:
each of the 9 taps is a matmul with a diagonal 128x128 weight matrix (per-channel tap
weight) against a shifted view of the padded image, accumulated in PSUM. All BN/conv
parameters are folded on the host into per-channel affine constants; the sigmoid of
the fusion gate is computed as 0.5+0.5*tanh(0.5*z) and folded into the final combine
so a single ACT table set (exp/tanh/relu) serves the whole kernel.
"""
import os
import sys

import numpy as np

sys.path.insert(0, '/opt/trn_rl_repo')

import concourse.bass as bass  # noqa: E402,F401
import concourse.tile as tile  # noqa: E402
from concourse import bacc, mybir  # noqa: E402
from concourse.bass_utils import run_bass_kernel_spmd  # noqa: E402

F32 = mybir.dt.float32
AF = mybir.ActivationFunctionType
OP = mybir.AluOpType

EPS = 1e-5
C, G, KK = 64, 8, 3
H = W = 128
NB = 8           # batch / cores
RLOW = 4         # bottleneck dim C//16
NPIX = H * W
NCHUNK = 16      # 512-pixel (4-row) chunks
CH = 512

# ---------------------------------------------------------------- host folding

def _fold_bn(p):
    g, b, m, v = [np.asarray(a, np.float32) for a in p]
    s = g / np.sqrt(v + EPS)
    return s, b - m * s


def _host_consts(params):
    P = {k: np.asarray(v, np.float32) for k, v in params.items()}
    o = {}
    s_f, t_f = _fold_bn(P['bn_f'])
    Wf = P['conv_w'] * s_f[:, None]                      # (72, 64)
    wf_gap = np.zeros((128, 72), np.float32)
    wf_gap[:64] = Wf.T / NPIX
    wf_gap[64:] = Wf.T / NPIX
    o['wf_gap'] = wf_gap
    o['bf_bias'] = t_f.reshape(72, 1)

    def se(tag, wk, bk, bnk, w2k, b2k, bn2k):
        s, t = _fold_bn(P[bnk])
        W1 = P[wk] * s[:, None]                          # (4, 64)
        b1 = s * P[bk] + t
        lhs = np.zeros((128, RLOW), np.float32)
        lhs[:64] = W1.T / NPIX
        lhs[64:] = W1.T / NPIX
        o['se_%s1' % tag] = lhs
        o['b_%s1' % tag] = b1.reshape(RLOW, 1)
        s2, t2 = _fold_bn(P[bn2k])
        W2 = P[w2k] * s2[:, None]                        # (64, 4)
        b2 = s2 * P[b2k] + t2
        o['se_%s2' % tag] = np.ascontiguousarray(W2.T)   # (4, 64) lhsT
        o['b_%s2' % tag] = b2.reshape(64, 1)

    se('l', 'fcl_w', 'fcl_b', 'bn_l1', 'fc0_w', 'fc0_b', 'bn_l2')
    se('h', 'fch_w', 'fch_b', 'bn_h1', 'fc1_w', 'fc1_b', 'bn_h2')

    s1, t1 = _fold_bn(P['bn_fu1'])
    Wf1 = P['fu1_w'] * s1[:, None]                       # (4, 64)
    bf1 = s1 * P['fu1_b'] + t1
    s2, t2 = _fold_bn(P['bn_fu2'])
    Wf2 = P['fu2_w'] * s2[:, None]                       # (64, 4)
    bf2 = s2 * P['fu2_b'] + t2

    wfu1 = np.zeros((128, 8), np.float32)
    wfu1[:64, :4] = Wf1.T
    wfu1[64:, 4:] = Wf1.T
    o['wfu1'] = wfu1
    o['wfu1_c'] = np.ascontiguousarray(Wf1.T)            # (64, 4)
    o['bfu1_dup'] = np.concatenate([bf1, bf1]).reshape(8, 1)
    wfu2 = np.zeros((8, 128), np.float32)
    wfu2[:4, :64] = Wf2.T
    wfu2[4:, 64:] = Wf2.T
    o['wfu2'] = wfu2
    o['bfu2_half'] = 0.5 * np.concatenate([bf2, bf2]).reshape(128, 1)

    gidx = np.arange(72) // 9
    pidx = np.arange(72) % 9
    o['t9'] = (pidx[:, None] == np.arange(9)[None, :]).astype(np.float32)
    cg = np.arange(128) % 64 // 8
    o['cmap'] = (gidx[:, None] == cg[None, :]).astype(np.float32)
    o['gsum'] = (gidx[:, None] == np.arange(8)[None, :]).astype(np.float32)
    o['gbcast'] = (np.arange(8)[:, None] == gidx[None, :]).astype(np.float32)
    o['i128'] = np.eye(128, dtype=np.float32)
    o['ones64'] = np.ones((64, 1), np.float32)
    o['ones1_64'] = np.ones((1, 64), np.float32)
    o['tile2'] = np.concatenate([np.eye(64), np.eye(64)], axis=1).astype(np.float32)
    o['dup4'] = np.concatenate([np.eye(4), np.eye(4)], axis=1).astype(np.float32)
    return o


# ---------------------------------------------------------------- device build

def _build(const_shapes):
    nc = bacc.Bacc("TRN2", target_bir_lowering=False, debug=False)
    x_in = nc.dram_tensor("x_in", [C, H, W], F32, kind="ExternalInput")
    y_out = nc.dram_tensor("y_out", [C, H, W], F32, kind="ExternalOutput")
    cin = {k: nc.dram_tensor(k, list(v), F32, kind="ExternalInput")
           for k, v in const_shapes.items()}

    with tile.TileContext(nc) as tc:
        _emit(tc, nc, x_in, y_out, cin)
    nc.compile()
    return nc


def _emit(tc, nc, x_in, y_out, cin):
    import contextlib
    ctx = contextlib.ExitStack()
    with ctx:
        big = ctx.enter_context(tc.tile_pool(name="big", bufs=1))
        small = ctx.enter_context(tc.tile_pool(name="small", bufs=1))
        work = ctx.enter_context(tc.tile_pool(name="work", bufs=3))
        outp = ctx.enter_context(tc.tile_pool(name="outp", bufs=2))

        # ---- constants to SBUF
        cs = {}
        for k, t in cin.items():
            cs[k] = small.tile(list(t.shape), F32, tag=k, name="c_" + k)
            nc.sync.dma_start(cs[k][:], t.ap())

        # ---- warm the ACT table set (exp/tanh/relu all in exp_and_others)
        warm = small.tile([1, 1], F32, tag="warm")
        nc.vector.memset(warm[:], 0.0)
        nc.scalar.activation(warm[:], warm[:], AF.Exp)

        # ---- warm the PE HAM clock gate during the input DMA (~4us of
        # dummy matmuls so the real conv starts at 2.4 GHz)
        with tc.tile_pool(name="pwarm", bufs=1, space="PSUM") as pwarm:
            pw = pwarm.tile([128, 128], F32, tag="pw")
            for _ in range(40):
                nc.tensor.matmul(pw[:], cs['i128'][:], cs['i128'][:])

        # ---- padded input image
        xp = big.tile([128, 66, 130], F32, tag="xp")
        xa = x_in.ap()
        nc.sync.dma_start(xp[0:64, 1:66, 1:129], xa[:, 0:65, :])
        nc.sync.dma_start(xp[64:128, 0:65, 1:129], xa[:, 63:128, :])
        nc.sync.dma_start(xp[0:64, 0:1, 1:129], xa[:, 1:2, :])
        nc.sync.dma_start(xp[64:128, 65:66, 1:129], xa[:, 126:127, :])
        nc.vector.tensor_copy(xp[:, :, 0:1], xp[:, :, 2:3])
        nc.vector.tensor_copy(xp[:, :, 129:130], xp[:, :, 127:128])

        x_view = xp[:, 1:65, 1:129]      # unpadded image view (128, 64, 128)

        # ---- global average pool of x (per-partition row sums), split
        # between DVE (tensor_reduce) and ACT (copy with accum_out)
        xsum = small.tile([128, 1], F32, tag="xsum")
        xsum_b = small.tile([128, 1], F32, tag="xsum_b")
        scrap = big.tile([128, 24, 128], F32, tag="scrap")
        nc.vector.tensor_reduce(xsum[:], xp[:, 1:41, 1:129],
                                mybir.AxisListType.XY, OP.add)
        nc.scalar.activation(scrap[:], xp[:, 41:65, 1:129], AF.Copy,
                             accum_out=xsum_b[:])
        nc.vector.tensor_tensor(xsum[:], xsum[:], xsum_b[:], OP.add)

        # ---- dynamic filter -> 9 diagonal weight matrices
        dmats = small.tile([128, 9, 128], F32, tag="dmats")
        with tc.tile_pool(name="pfilt", bufs=1, space="PSUM") as pfilt:
            pf = pfilt.tile([72, 1], F32, tag="pf")
            nc.tensor.matmul(pf[:], cs['wf_gap'][:], xsum[:])
            efilt = small.tile([72, 1], F32, tag="efilt")
            nc.scalar.activation(efilt[:], pf[:], AF.Exp, bias=cs['bf_bias'][:])
            pg = pfilt.tile([8, 1], F32, tag="pg")
            nc.tensor.matmul(pg[:], cs['gsum'][:], efilt[:])
            gs = small.tile([8, 1], F32, tag="gs")
            nc.scalar.copy(gs[:], pg[:])
            rcp = small.tile([8, 1], F32, tag="rcp")
            nc.vector.reciprocal(rcp[:], gs[:])
            prb = pfilt.tile([72, 1], F32, tag="prb")
            nc.tensor.matmul(prb[:], cs['gbcast'][:], rcp[:])
            filt = small.tile([72, 1], F32, tag="filt")
            nc.vector.tensor_mul(filt[:], efilt[:], prb[:])
            fdiag = small.tile([72, 9], F32, tag="fdiag")
            nc.vector.tensor_scalar_mul(fdiag[:], cs['t9'][:], filt[:])
            pwall = pfilt.tile([128, 9], F32, tag="pwall")
            nc.tensor.matmul(pwall[:], cs['cmap'][:], fdiag[:])
            wall = small.tile([128, 9], F32, tag="wall")
            nc.scalar.copy(wall[:], pwall[:])
            for p in range(9):
                nc.vector.tensor_scalar_mul(dmats[:, p, :], cs['i128'][:],
                                            wall[:, p:p + 1])

        # ---- 9-tap dynamic conv on the tensor engine
        L = big.tile([128, NCHUNK, CH], F32, tag="L")
        lsums = small.tile([128, NCHUNK], F32, tag="lsums")
        NPE = 7   # taps 0..6 on the tensor engine, taps 7..8 on DVE
        with tc.tile_pool(name="pconv", bufs=3, space="PSUM") as pconv:
            for k in range(NCHUNK):
                pl = pconv.tile([128, CH], F32, tag="pl")
                for p in range(NPE):
                    di, dj = divmod(p, 3)
                    rhs = xp[:, 4 * k + di:4 * k + di + 4, dj:dj + 128]
                    nc.tensor.matmul(pl[:], dmats[:, p, :], rhs,
                                     start=(p == 0), stop=(p == NPE - 1))
                for p in range(NPE, 9):
                    di, dj = divmod(p, 3)
                    rhs = xp[:, 4 * k + di:4 * k + di + 4, dj:dj + 128]
                    nc.vector.scalar_tensor_tensor(
                        pl[:], rhs, wall[:, p:p + 1], pl[:], OP.mult, OP.add)
                nc.scalar.activation(L[:, k, :], pl[:], AF.Copy,
                                     accum_out=lsums[:, k:k + 1])

        # ---- SE attention chain (tiny)
        cf = small.tile([128, 10], F32, tag="cf")
        dmS = small.tile([128, 2, 128], F32, tag="dmS")
        dmD = small.tile([128, 2, 128], F32, tag="dmD")
        w1s = small.tile([128, 2, 8], F32, tag="w1s")
        vbias = small.tile([8, 1], F32, tag="vbias")
        with tc.tile_pool(name="pse", bufs=1, space="PSUM") as pse:
            lsum = small.tile([128, 1], F32, tag="lsum")
            nc.vector.tensor_reduce(lsum[:], lsums[:], mybir.AxisListType.X, OP.add)
            ohsum = small.tile([128, 1], F32, tag="ohsum")
            nc.vector.tensor_tensor(ohsum[:], xsum[:], lsum[:], OP.subtract)

            pv1 = pse.tile([RLOW, 2], F32, tag="pv1")
            nc.tensor.matmul(pv1[:, 0:1], cs['se_l1'][:], lsum[:])
            nc.tensor.matmul(pv1[:, 1:2], cs['se_h1'][:], ohsum[:])
            v1l = small.tile([RLOW, 1], F32, tag="v1l")
            v1h = small.tile([RLOW, 1], F32, tag="v1h")
            nc.scalar.activation(v1l[:], pv1[:, 0:1], AF.Relu, bias=cs['b_l1'][:])
            nc.scalar.activation(v1h[:], pv1[:, 1:2], AF.Relu, bias=cs['b_h1'][:])

            ppre = pse.tile([64, 2], F32, tag="ppre")
            nc.tensor.matmul(ppre[:, 0:1], cs['se_l2'][:], v1l[:])
            nc.tensor.matmul(ppre[:, 1:2], cs['se_h2'][:], v1h[:])
            pre_l = small.tile([64, 1], F32, tag="pre_l")
            nc.scalar.activation(pre_l[:], ppre[:, 0:1], AF.Identity,
                                 bias=cs['b_l2'][:])
            ee = small.tile([64, 2], F32, tag="ee")
            nc.scalar.activation(ee[:, 0:1], ppre[:, 0:1], AF.Exp, bias=cs['b_l2'][:])
            nc.scalar.activation(ee[:, 1:2], ppre[:, 1:2], AF.Exp, bias=cs['b_h2'][:])

            psm = pse.tile([1, 2], F32, tag="psm")
            nc.tensor.matmul(psm[:], cs['ones64'][:], ee[:])
            sm = small.tile([1, 2], F32, tag="sm")
            nc.scalar.copy(sm[:], psm[:])
            rcp2 = small.tile([1, 2], F32, tag="rcp2")
            nc.vector.reciprocal(rcp2[:], sm[:])
            prb2 = pse.tile([64, 2], F32, tag="prb2")
            nc.tensor.matmul(prb2[:], cs['ones1_64'][:], rcp2[:])
            att = small.tile([64, 2], F32, tag="att")
            nc.vector.tensor_mul(att[:], ee[:], prb2[:])

            p128 = pse.tile([128, 3], F32, tag="p128")
            nc.tensor.matmul(p128[:, 0:2], cs['tile2'][:], att[:])
            nc.tensor.matmul(p128[:, 2:3], cs['tile2'][:], pre_l[:])
            attv = small.tile([128, 3], F32, tag="attv")
            nc.scalar.copy(attv[:], p128[:])
            a_l = attv[:, 0:1]
            a_h = attv[:, 1:2]
            p_l = attv[:, 2:3]

            # per-channel coefficient vectors
            d_ = cf[:, 0:1]       # a_l - a_h
            bx = cf[:, 1:2]       # 1 + a_h
            aSp = cf[:, 2:3]      # 1.5 d
            bSp = cf[:, 3:4]      # 1.5 bx
            cSp = cf[:, 4:5]      # 1.5 p_l
            aDp = cf[:, 5:6]      # 1 + 0.5 (a_l + a_h)
            bDp = cf[:, 6:7]      # -0.5 bx
            cDp = cf[:, 7:8]      # 0.5 p_l
            ssum = cf[:, 8:9]     # a_l + a_h
            nc.vector.tensor_tensor(d_, a_l, a_h, OP.subtract)
            nc.vector.tensor_scalar_add(bx, a_h, 1.0)
            nc.vector.tensor_scalar_mul(aSp, d_, 1.5)
            nc.vector.tensor_scalar_mul(bSp, bx, 1.5)
            nc.vector.tensor_scalar_mul(cSp, p_l, 1.5)
            nc.vector.tensor_tensor(ssum, a_l, a_h, OP.add)
            nc.vector.tensor_scalar(aDp, ssum, 0.5, 1.0, OP.mult, OP.add)
            nc.vector.tensor_scalar_mul(bDp, bx, -0.5)
            nc.vector.tensor_scalar_mul(cDp, p_l, 0.5)

            nc.vector.tensor_scalar_mul(dmS[:, 0, :], cs['i128'][:], aSp)
            nc.vector.tensor_scalar_mul(dmS[:, 1, :], cs['i128'][:], bSp)
            nc.vector.tensor_scalar_mul(dmD[:, 0, :], cs['i128'][:], aDp)
            nc.vector.tensor_scalar_mul(dmD[:, 1, :], cs['i128'][:], bDp)
            nc.vector.tensor_scalar_mul(w1s[:, 0, :], cs['wfu1'][:], d_)
            nc.vector.tensor_scalar_mul(w1s[:, 1, :], cs['wfu1'][:], bx)

            pcb = pse.tile([RLOW, 1], F32, tag="pcb")
            nc.tensor.matmul(pcb[:], cs['wfu1_c'][:], pre_l[:])
            cb = small.tile([RLOW, 1], F32, tag="cb")
            nc.scalar.copy(cb[:], pcb[:])
            pvb = pse.tile([8, 1], F32, tag="pvb")
            nc.tensor.matmul(pvb[:], cs['dup4'][:], cb[:])
            nc.scalar.activation(vbias[:], pvb[:], AF.Identity,
                                 bias=cs['bfu1_dup'][:])

        cSp = cf[:, 4:5]
        cDp = cf[:, 7:8]

        # ---- final phase: per 512-pixel chunk
        with tc.tile_pool(name="pfin", bufs=2, space="PSUM") as pfin:
            och = None
            for k in range(NCHUNK):
                lch = L[:, k, :]
                xch = xp[:, 1 + 4 * k:1 + 4 * k + 4, 1:129]
                pS = pfin.tile([128, CH], F32, tag="pS")
                nc.tensor.matmul(pS[:], dmS[:, 0, :], lch, start=True, stop=False)
                nc.tensor.matmul(pS[:], dmS[:, 1, :], xch, start=False, stop=True)
                pD = pfin.tile([128, CH], F32, tag="pD")
                nc.tensor.matmul(pD[:], dmD[:, 0, :], lch, start=True, stop=False)
                nc.tensor.matmul(pD[:], dmD[:, 1, :], xch, start=False, stop=True)
                pc1 = pfin.tile([8, CH], F32, tag="pc1")
                nc.tensor.matmul(pc1[:], w1s[:, 0, :], lch, start=True, stop=False)
                nc.tensor.matmul(pc1[:], w1s[:, 1, :], xch, start=False, stop=True)
                vch = work.tile([8, CH], F32, tag="vch")
                nc.scalar.activation(vch[:], pc1[:], AF.Relu, bias=vbias[:])
                pc2 = pfin.tile([128, CH], F32, tag="pc2")
                nc.tensor.matmul(pc2[:], cs['wfu2'][:], vch[:])
                tch = work.tile([128, CH], F32, tag="tch")
                nc.scalar.activation(tch[:], pc2[:], AF.Tanh,
                                     bias=cs['bfu2_half'][:], scale=0.5)
                dt = work.tile([128, CH], F32, tag="dt")
                nc.vector.scalar_tensor_tensor(dt[:], pD[:], cDp, tch[:],
                                               OP.add, OP.mult)
                if k % 4 == 0:
                    och = outp.tile([128, 4, CH], F32, tag="och")
                nc.vector.scalar_tensor_tensor(och[:, k % 4, :], pS[:], cSp, dt[:],
                                               OP.add, OP.add)
                if k % 4 == 3:
                    q = k // 4
                    yv = y_out.ap().rearrange("c (s r) w -> s c r w", s=2)
                    nc.sync.dma_start(
                        yv[:, :, 16 * q:16 * q + 16, :],
                        och[:].rearrange("p a b -> p (a b)"))


# ---------------------------------------------------------------- entry point

_CACHE = {}


def kernel(x, params):
    x = np.asarray(x, np.float32)
    consts = _host_consts(params)
    if 'nc' not in _CACHE:
        _CACHE['nc'] = _build({k: v.shape for k, v in consts.items()})
    nc = _CACHE['nc']
    in_maps = []
    for i in range(NB):
        m = {'x_in': np.ascontiguousarray(x[i])}
        m.update(consts)
        in_maps.append(m)
    res = run_bass_kernel_spmd(nc, in_maps, core_ids=list(range(NB)), trace=False)
    if os.environ.get('KERNEL_TRACE'):
        print("HW exec time: %d ns" % estimate_time_ns())
    return np.stack([r['y_out'] for r in res.results])


def estimate_time_ns(trace_path=None):
    """Cost-model (TimelineSim) estimate of the per-core kernel duration."""
    nc = _CACHE['nc']
    from concourse.timeline_sim import TimelineSim
    tl = TimelineSim(nc, trace=bool(trace_path))
    dur = tl.simulate()
    if trace_path:
        try:
            tl.write_trace(trace_path)
        except Exception:
            pass
    return int(dur)


# revision 31
# speedup vs baseline: 2.9890x; 2.9890x over previous
"""Trainium2 Bass kernel for the FFE module.

Sharding: pure data parallelism — sample i of the batch (N=8) runs on core i.

Per-core layout: 128 SBUF partitions = (row-half hh in {0,1}) x (channel c in 0..63),
partition index hh*64 + c. Each partition holds a reflection-padded sub-image of its
channel: 66 rows x 130 cols (its 64 output rows plus a 1-row/1-col halo).

The dynamic 3x3 depthwise conv (softmax filter from GAP) runs on the tensor engine:
each of the 9 taps is a matmul with a diagonal 128x128 weight matrix (per-channel tap
weight) against a shifted view of the padded image, accumulated in PSUM. All BN/conv
parameters are folded on the host into per-channel affine constants; the sigmoid of
the fusion gate is computed as 0.5+0.5*tanh(0.5*z) and folded into the final combine
so a single ACT table set (exp/tanh/relu) serves the whole kernel.
"""
import os
import sys

import numpy as np

sys.path.insert(0, '/opt/trn_rl_repo')

import concourse.bass as bass  # noqa: E402,F401
import concourse.tile as tile  # noqa: E402
from concourse import bacc, mybir  # noqa: E402
from concourse.bass_utils import run_bass_kernel_spmd  # noqa: E402

F32 = mybir.dt.float32
F32R = mybir.dt.float32r
AF = mybir.ActivationFunctionType
OP = mybir.AluOpType

EPS = 1e-5
C, G, KK = 64, 8, 3
H = W = 128
NB = 8           # batch / cores
RLOW = 4         # bottleneck dim C//16
NPIX = H * W
NCHUNK = 16      # 512-pixel (4-row) chunks
CH = 512

# ---------------------------------------------------------------- host folding

def _fold_bn(p):
    g, b, m, v = [np.asarray(a, np.float32) for a in p]
    s = g / np.sqrt(v + EPS)
    return s, b - m * s


def _host_consts(params):
    P = {k: np.asarray(v, np.float32) for k, v in params.items()}
    o = {}
    s_f, t_f = _fold_bn(P['bn_f'])
    Wf = P['conv_w'] * s_f[:, None]                      # (72, 64)
    wf_gap = np.zeros((128, 72), np.float32)
    wf_gap[:64] = Wf.T / NPIX
    wf_gap[64:] = Wf.T / NPIX
    o['wf_gap'] = wf_gap
    o['bf_bias'] = t_f.reshape(72, 1)

    def se(tag, wk, bk, bnk, w2k, b2k, bn2k):
        s, t = _fold_bn(P[bnk])
        W1 = P[wk] * s[:, None]                          # (4, 64)
        b1 = s * P[bk] + t
        lhs = np.zeros((128, RLOW), np.float32)
        lhs[:64] = W1.T / NPIX
        lhs[64:] = W1.T / NPIX
        o['se_%s1' % tag] = lhs
        o['b_%s1' % tag] = b1.reshape(RLOW, 1)
        s2, t2 = _fold_bn(P[bn2k])
        W2 = P[w2k] * s2[:, None]                        # (64, 4)
        b2 = s2 * P[b2k] + t2
        o['se_%s2' % tag] = np.ascontiguousarray(W2.T)   # (4, 64) lhsT
        o['b_%s2' % tag] = b2.reshape(64, 1)

    se('l', 'fcl_w', 'fcl_b', 'bn_l1', 'fc0_w', 'fc0_b', 'bn_l2')
    se('h', 'fch_w', 'fch_b', 'bn_h1', 'fc1_w', 'fc1_b', 'bn_h2')

    s1, t1 = _fold_bn(P['bn_fu1'])
    Wf1 = P['fu1_w'] * s1[:, None]                       # (4, 64)
    bf1 = s1 * P['fu1_b'] + t1
    s2, t2 = _fold_bn(P['bn_fu2'])
    Wf2 = P['fu2_w'] * s2[:, None]                       # (64, 4)
    bf2 = s2 * P['fu2_b'] + t2

    wfu1 = np.zeros((128, 8), np.float32)
    wfu1[:64, :4] = Wf1.T
    wfu1[64:, 4:] = Wf1.T
    o['wfu1'] = wfu1
    o['wfu1_c'] = np.ascontiguousarray(Wf1.T)            # (64, 4)
    o['bfu1_dup'] = np.concatenate([bf1, bf1]).reshape(8, 1)
    wfu2 = np.zeros((8, 128), np.float32)
    wfu2[:4, :64] = Wf2.T
    wfu2[4:, 64:] = Wf2.T
    o['wfu2'] = wfu2
    o['bfu2_half'] = 0.5 * np.concatenate([bf2, bf2]).reshape(128, 1)

    gidx = np.arange(72) // 9
    pidx = np.arange(72) % 9
    o['t9'] = (pidx[:, None] == np.arange(9)[None, :]).astype(np.float32)
    cg = np.arange(128) % 64 // 8
    o['cmap'] = (gidx[:, None] == cg[None, :]).astype(np.float32)
    o['gsum'] = (gidx[:, None] == np.arange(8)[None, :]).astype(np.float32)
    o['gbcast'] = (np.arange(8)[:, None] == gidx[None, :]).astype(np.float32)
    o['i128'] = np.eye(128, dtype=np.float32)
    o['ones64'] = np.ones((64, 1), np.float32)
    o['ones1_64'] = np.ones((1, 64), np.float32)
    o['tile2'] = np.concatenate([np.eye(64), np.eye(64)], axis=1).astype(np.float32)
    o['dup4'] = np.concatenate([np.eye(4), np.eye(4)], axis=1).astype(np.float32)

    # pack everything into one (128, N) blob — a single DMA instead of ~24
    layout = {}
    col = 0
    for k in sorted(o):
        r, c = o[k].shape
        layout[k] = (r, col, c)
        col += c
    blob = np.zeros((128, col), np.float32)
    for k, (r, c0, c) in layout.items():
        blob[:r, c0:c0 + c] = o[k]
    return {'cblob': blob}, layout


# ---------------------------------------------------------------- device build

def _build(blob_shape, layout):
    nc = bacc.Bacc("TRN2", target_bir_lowering=False, debug=False)
    x_in = nc.dram_tensor("x_in", [C, H, W], F32R, kind="ExternalInput")
    y_out = nc.dram_tensor("y_out", [C, H, W], F32, kind="ExternalOutput")
    cblob = nc.dram_tensor("cblob", list(blob_shape), F32, kind="ExternalInput")

    with tile.TileContext(nc) as tc:
        _emit(tc, nc, x_in, y_out, cblob, layout)
    nc.compile()
    return nc


def _emit(tc, nc, x_in, y_out, cblob, layout):
    import contextlib
    ctx = contextlib.ExitStack()
    with ctx:
        big = ctx.enter_context(tc.tile_pool(name="big", bufs=1))
        small = ctx.enter_context(tc.tile_pool(name="small", bufs=1))
        work = ctx.enter_context(tc.tile_pool(name="work", bufs=3))
        outp = ctx.enter_context(tc.tile_pool(name="outp", bufs=2))

        # ---- constants to SBUF (one packed DMA, then views)
        ncol = cblob.shape[1]
        blob = small.tile([128, ncol], F32, tag="cblob", name="cblob_sb")
        nc.sync.dma_start(blob[:], cblob.ap())
        cs = {k: blob[0:r, c0:c0 + c] for k, (r, c0, c) in layout.items()}

        # ---- warm the ACT table set (exp/tanh/relu all in exp_and_others)
        warm = small.tile([1, 1], F32, tag="warm")
        nc.vector.memset(warm[:], 0.0)
        nc.scalar.activation(warm[:], warm[:], AF.Exp)

        # ---- warm the PE HAM clock gate during the input DMA (~4us of
        # dummy matmuls so the real conv starts at 2.4 GHz)
        with tc.tile_pool(name="pwarm", bufs=1, space="PSUM") as pwarm:
            pw = pwarm.tile([128, 128], F32, tag="pw")
            for _ in range(10):
                nc.tensor.matmul(pw[:], cs['i128'][:], cs['i128'][:])

        # ---- padded input image. Both row-halves load together: a 4-dim
        # source AP walks (half, channel, row, col) so partition (hh,c)
        # receives x[c, hh*64 + r, :]. Two row-blocks let the GAP reduce
        # start on block 1 while block 2 is still in flight.
        xp = big.tile([128, 66, 130], F32R, tag="xp")
        xa = x_in.ap()
        nc.sync.dma_start(xp[0:64, 1:33, 1:129], xa[:, 0:32, :])
        nc.scalar.dma_start(xp[64:128, 1:33, 1:129], xa[:, 64:96, :])
        nc.sync.dma_start(xp[0:64, 33:65, 1:129], xa[:, 32:64, :])
        nc.scalar.dma_start(xp[64:128, 33:65, 1:129], xa[:, 96:128, :])
        # halo rows: local 0 is x row 1 (reflect) for the top half and x row
        # 63 for the bottom half; local 65 is x row 64 / x row 126 (reflect).
        xf = x_in.ap().rearrange("c h w -> c (h w)")
        halo0 = bass.AP(tensor=xf.tensor, offset=xf.offset + 1 * W,
                        ap=[[62 * W, 2]] + [[xf.ap[0][0], 64], [1, W]])
        halo1 = bass.AP(tensor=xf.tensor, offset=xf.offset + 64 * W,
                        ap=[[62 * W, 2]] + [[xf.ap[0][0], 64], [1, W]])
        nc.sync.dma_start(xp[:, 0:1, 1:129], halo0)
        nc.scalar.dma_start(xp[:, 65:66, 1:129], halo1)
        nc.vector.tensor_copy(xp[:, :, 0:1], xp[:, :, 2:3])
        nc.vector.tensor_copy(xp[:, :, 129:130], xp[:, :, 127:128])

        x_view = xp[:, 1:65, 1:129]      # unpadded image view (128, 64, 128)

        # ---- global average pool of x: partial reduces per DMA block,
        # split between DVE (tensor_reduce) and ACT (copy with accum_out)
        xsum = small.tile([128, 4], F32, tag="xsum")
        scrap = big.tile([128, 16, 128], F32, tag="scrap")
        nc.vector.tensor_reduce(xsum[:, 0:1], xp[:, 1:17, 1:129],
                                mybir.AxisListType.XY, OP.add)
        nc.scalar.activation(scrap[:], xp[:, 17:33, 1:129], AF.Copy,
                             accum_out=xsum[:, 1:2])
        nc.vector.tensor_reduce(xsum[:, 2:3], xp[:, 33:49, 1:129],
                                mybir.AxisListType.XY, OP.add)
        nc.scalar.activation(scrap[:], xp[:, 49:65, 1:129], AF.Copy,
                             accum_out=xsum[:, 3:4])
        xsum_t = small.tile([128, 1], F32, tag="xsum_t")
        nc.vector.tensor_reduce(xsum_t[:], xsum[:], mybir.AxisListType.X, OP.add)
        xsum = xsum_t

        # ---- dynamic filter -> 9 diagonal weight matrices
        dmats = small.tile([128, 9, 128], F32R, tag="dmats")
        with tc.tile_pool(name="pfilt", bufs=1, space="PSUM") as pfilt:
            pf = pfilt.tile([72, 1], F32, tag="pf")
            nc.tensor.matmul(pf[:], cs['wf_gap'][:], xsum[:])
            efilt = small.tile([72, 1], F32, tag="efilt")
            nc.scalar.activation(efilt[:], pf[:], AF.Exp, bias=cs['bf_bias'][:])
            pg = pfilt.tile([8, 1], F32, tag="pg")
            nc.tensor.matmul(pg[:], cs['gsum'][:], efilt[:])
            gs = small.tile([8, 1], F32, tag="gs")
            nc.scalar.copy(gs[:], pg[:])
            rcp = small.tile([8, 1], F32, tag="rcp")
            nc.vector.reciprocal(rcp[:], gs[:])
            prb = pfilt.tile([72, 1], F32, tag="prb")
            nc.tensor.matmul(prb[:], cs['gbcast'][:], rcp[:])
            filt = small.tile([72, 1], F32, tag="filt")
            nc.vector.tensor_mul(filt[:], efilt[:], prb[:])
            fdiag = small.tile([72, 9], F32, tag="fdiag")
            nc.vector.tensor_scalar_mul(fdiag[:], cs['t9'][:], filt[:])
            pwall = pfilt.tile([128, 9], F32, tag="pwall")
            nc.tensor.matmul(pwall[:], cs['cmap'][:], fdiag[:])
            wall = small.tile([128, 9], F32, tag="wall")
            nc.scalar.copy(wall[:], pwall[:])
            for p in range(9):
                nc.vector.tensor_scalar_mul(dmats[:, p, :], cs['i128'][:],
                                            wall[:, p:p + 1])

        # ---- 9-tap dynamic conv on the tensor engine
        L = big.tile([128, NCHUNK, CH], F32R, tag="L")
        lsums = small.tile([128, NCHUNK], F32, tag="lsums")
        NPE = 7   # taps 0..6 on the tensor engine, taps 7..8 on DVE
        with tc.tile_pool(name="pconv", bufs=3, space="PSUM") as pconv:
            for k in range(NCHUNK):
                pl = pconv.tile([128, CH], F32, tag="pl")
                for p in range(NPE):
                    di, dj = divmod(p, 3)
                    rhs = xp[:, 4 * k + di:4 * k + di + 4, dj:dj + 128]
                    nc.tensor.matmul(pl[:], dmats[:, p, :], rhs,
                                     start=(p == 0), stop=(p == NPE - 1))
                for p in range(NPE, 9):
                    di, dj = divmod(p, 3)
                    rhs = xp[:, 4 * k + di:4 * k + di + 4, dj:dj + 128]
                    nc.vector.scalar_tensor_tensor(
                        pl[:], rhs, wall[:, p:p + 1], pl[:], OP.mult, OP.add)
                nc.scalar.activation(L[:, k, :], pl[:], AF.Copy,
                                     accum_out=lsums[:, k:k + 1])

        # ---- SE attention chain (tiny)
        cf = small.tile([128, 10], F32, tag="cf")
        dmS = small.tile([128, 2, 128], F32R, tag="dmS")
        dmD = small.tile([128, 2, 128], F32R, tag="dmD")
        w1s = small.tile([128, 2, 8], F32R, tag="w1s")
        wfu2r = small.tile([8, 128], F32R, tag="wfu2r")
        nc.vector.tensor_copy(wfu2r[:], cs['wfu2'][:])
        vbias = small.tile([8, 1], F32, tag="vbias")
        with tc.tile_pool(name="pse", bufs=1, space="PSUM") as pse:
            lsum = small.tile([128, 1], F32, tag="lsum")
            nc.vector.tensor_reduce(lsum[:], lsums[:], mybir.AxisListType.X, OP.add)
            ohsum = small.tile([128, 1], F32, tag="ohsum")
            nc.vector.tensor_tensor(ohsum[:], xsum[:], lsum[:], OP.subtract)

            pv1 = pse.tile([RLOW, 2], F32, tag="pv1")
            nc.tensor.matmul(pv1[:, 0:1], cs['se_l1'][:], lsum[:])
            nc.tensor.matmul(pv1[:, 1:2], cs['se_h1'][:], ohsum[:])
            v1l = small.tile([RLOW, 1], F32, tag="v1l")
            v1h = small.tile([RLOW, 1], F32, tag="v1h")
            nc.scalar.activation(v1l[:], pv1[:, 0:1], AF.Relu, bias=cs['b_l1'][:])
            nc.scalar.activation(v1h[:], pv1[:, 1:2], AF.Relu, bias=cs['b_h1'][:])

            ppre = pse.tile([64, 2], F32, tag="ppre")
            nc.tensor.matmul(ppre[:, 0:1], cs['se_l2'][:], v1l[:])
            nc.tensor.matmul(ppre[:, 1:2], cs['se_h2'][:], v1h[:])
            pre_l = small.tile([64, 1], F32, tag="pre_l")
            nc.scalar.activation(pre_l[:], ppre[:, 0:1], AF.Identity,
                                 bias=cs['b_l2'][:])
            ee = small.tile([64, 2], F32, tag="ee")
            nc.scalar.activation(ee[:, 0:1], ppre[:, 0:1], AF.Exp, bias=cs['b_l2'][:])
            nc.scalar.activation(ee[:, 1:2], ppre[:, 1:2], AF.Exp, bias=cs['b_h2'][:])

            psm = pse.tile([1, 2], F32, tag="psm")
            nc.tensor.matmul(psm[:], cs['ones64'][:], ee[:])
            sm = small.tile([1, 2], F32, tag="sm")
            nc.scalar.copy(sm[:], psm[:])
            rcp2 = small.tile([1, 2], F32, tag="rcp2")
            nc.vector.reciprocal(rcp2[:], sm[:])
            prb2 = pse.tile([64, 2], F32, tag="prb2")
            nc.tensor.matmul(prb2[:], cs['ones1_64'][:], rcp2[:])
            att = small.tile([64, 2], F32, tag="att")
            nc.vector.tensor_mul(att[:], ee[:], prb2[:])

            p128 = pse.tile([128, 3], F32, tag="p128")
            nc.tensor.matmul(p128[:, 0:2], cs['tile2'][:], att[:])
            nc.tensor.matmul(p128[:, 2:3], cs['tile2'][:], pre_l[:])
            attv = small.tile([128, 3], F32, tag="attv")
            nc.scalar.copy(attv[:], p128[:])
            a_l = attv[:, 0:1]
            a_h = attv[:, 1:2]
            p_l = attv[:, 2:3]

            # per-channel coefficient vectors
            d_ = cf[:, 0:1]       # a_l - a_h
            bx = cf[:, 1:2]       # 1 + a_h
            aSp = cf[:, 2:3]      # 1.5 d
            bSp = cf[:, 3:4]      # 1.5 bx
            cSp = cf[:, 4:5]      # 1.5 p_l
            aDp = cf[:, 5:6]      # 1 + 0.5 (a_l + a_h)
            bDp = cf[:, 6:7]      # -0.5 bx
            cDp = cf[:, 7:8]      # 0.5 p_l
            ssum = cf[:, 8:9]     # a_l + a_h
            nc.vector.tensor_tensor(d_, a_l, a_h, OP.subtract)
            nc.vector.tensor_scalar_add(bx, a_h, 1.0)
            nc.vector.tensor_scalar_mul(aSp, d_, 1.5)
            nc.vector.tensor_scalar_mul(bSp, bx, 1.5)
            nc.vector.tensor_scalar_mul(cSp, p_l, 1.5)
            nc.vector.tensor_tensor(ssum, a_l, a_h, OP.add)
            nc.vector.tensor_scalar(aDp, ssum, 0.5, 1.0, OP.mult, OP.add)
            nc.vector.tensor_scalar_mul(bDp, bx, -0.5)
            nc.vector.tensor_scalar_mul(cDp, p_l, 0.5)

            nc.vector.tensor_scalar_mul(dmS[:, 0, :], cs['i128'][:], aSp)
            nc.vector.tensor_scalar_mul(dmS[:, 1, :], cs['i128'][:], bSp)
            nc.vector.tensor_scalar_mul(dmD[:, 0, :], cs['i128'][:], aDp)
            nc.vector.tensor_scalar_mul(dmD[:, 1, :], cs['i128'][:], bDp)
            nc.vector.tensor_scalar_mul(w1s[:, 0, :], cs['wfu1'][:], d_)
            nc.vector.tensor_scalar_mul(w1s[:, 1, :], cs['wfu1'][:], bx)

            pcb = pse.tile([RLOW, 1], F32, tag="pcb")
            nc.tensor.matmul(pcb[:], cs['wfu1_c'][:], pre_l[:])
            cb = small.tile([RLOW, 1], F32, tag="cb")
            nc.scalar.copy(cb[:], pcb[:])
            pvb = pse.tile([8, 1], F32, tag="pvb")
            nc.tensor.matmul(pvb[:], cs['dup4'][:], cb[:])
            nc.scalar.activation(vbias[:], pvb[:], AF.Identity,
                                 bias=cs['bfu1_dup'][:])

        cSp = cf[:, 4:5]
        cDp = cf[:, 7:8]

        # ---- final phase: per 512-pixel chunk
        with tc.tile_pool(name="pfin", bufs=2, space="PSUM") as pfin:
            och = None
            for k in range(NCHUNK):
                lch = L[:, k, :]
                xch = xp[:, 1 + 4 * k:1 + 4 * k + 4, 1:129]
                pS = pfin.tile([128, CH], F32, tag="pS")
                nc.tensor.matmul(pS[:], dmS[:, 0, :], lch, start=True, stop=False)
                nc.tensor.matmul(pS[:], dmS[:, 1, :], xch, start=False, stop=True)
                pD = pfin.tile([128, CH], F32, tag="pD")
                nc.tensor.matmul(pD[:], dmD[:, 0, :], lch, start=True, stop=False)
                nc.tensor.matmul(pD[:], dmD[:, 1, :], xch, start=False, stop=True)
                pc1 = pfin.tile([8, CH], F32, tag="pc1")
                nc.tensor.matmul(pc1[:], w1s[:, 0, :], lch, start=True, stop=False)
                nc.tensor.matmul(pc1[:], w1s[:, 1, :], xch, start=False, stop=True)
                vch = work.tile([8, CH], F32R, tag="vch")
                nc.scalar.activation(vch[:], pc1[:], AF.Relu, bias=vbias[:])
                pc2 = pfin.tile([128, CH], F32, tag="pc2")
                nc.tensor.matmul(pc2[:], wfu2r[:], vch[:])
                tch = work.tile([128, CH], F32, tag="tch")
                nc.scalar.activation(tch[:], pc2[:], AF.Tanh,
                                     bias=cs['bfu2_half'][:], scale=0.5)
                dt = work.tile([128, CH], F32, tag="dt")
                nc.vector.scalar_tensor_tensor(dt[:], pD[:], cDp, tch[:],
                                               OP.add, OP.mult)
                if k % 2 == 0:
                    och = outp.tile([128, 2, CH], F32, tag="och")
                nc.vector.scalar_tensor_tensor(och[:, k % 2, :], pS[:], cSp, dt[:],
                                               OP.add, OP.add)
                if k % 2 == 1:
                    q = k // 2
                    yv = y_out.ap().rearrange("c (s r) w -> s c r w", s=2)
                    eng = nc.sync if q % 2 == 0 else nc.scalar
                    eng.dma_start(
                        yv[:, :, 8 * q:8 * q + 8, :],
                        och[:].rearrange("p a b -> p (a b)"))


# ---------------------------------------------------------------- entry point

_CACHE = {}


def kernel(x, params):
    x = np.asarray(x, np.float32)
    consts, layout = _host_consts(params)
    if 'nc' not in _CACHE:
        _CACHE['nc'] = _build(consts['cblob'].shape, layout)
    nc = _CACHE['nc']
    in_maps = []
    for i in range(NB):
        m = {'x_in': np.ascontiguousarray(x[i])}
        m.update(consts)
        in_maps.append(m)
    res = run_bass_kernel_spmd(nc, in_maps, core_ids=list(range(NB)), trace=False)
    if os.environ.get('KERNEL_TRACE'):
        print("HW exec time: %d ns" % estimate_time_ns())
    return np.stack([r['y_out'] for r in res.results])


def estimate_time_ns(trace_path=None):
    """Cost-model (TimelineSim) estimate of the per-core kernel duration."""
    nc = _CACHE['nc']
    from concourse.timeline_sim import TimelineSim
    tl = TimelineSim(nc, trace=bool(trace_path))
    dur = tl.simulate()
    if trace_path:
        try:
            tl.write_trace(trace_path)
        except Exception:
            pass
    return int(dur)
